# revision 58
# baseline (speedup 1.0000x reference)
"""BCP quantized linear SPMD kernel for 8 Trainium2 NeuronCores.

Computes y = x @ W_deq.T + bias where
  W_deq = ((W_q - zeros) * scales) * mu2[:,None] * mu1[None,:] * mask

Sharding: tensor-parallel along the output dim K (8192 -> 1024 rows/core).
x is replicated; the [64, 1024] per-core outputs are concatenated on the
host.

The host folds the entire dequant into an int8 recode of the weight:
  v[k,n]  = (W_q - zeros) * scales * mu2 * mask          (mu1 folds into x)
  d[k]    = max_n |v[k,n]| / 127
  e8[k,n] = rint(v[k,n] / d[k])                          (int8)
so on device y_raw = x' @ e8.T is a single f16 matmul over the int8
stream, and the host applies the per-row scale d[k] and bias to the
gathered output.

The int8 -> f16 up-conversion is spread over three producers so no
single resource binds (PE streaming floor is ~27.5 us):
  'c' tiles: SWDGE cast-DMA (2 B/elem SBUF writes),
  'v' tiles: raw int8 DMA + VectorE copy-cast (~0.62 us/tile),
  'a' tiles: raw int8 DMA + ScalarE copy-cast (~1.04 us/tile).
DMA chunks are >=4 tiles so per-partition rows are >=4 KB (small rows
are descriptor-generation-bound); casts run at 2-tile granularity.
Each tile t contributes two accumulating matmuls (PSUM [128, 512] x2,
rows 64..127 discarded) with lhsT the overlapping 128-column window
x'T[:, t*64 : t*64+128] (n permuted as n = p*64 + t), which keeps Fast
Weight Load enabled (~215 ns/MM instead of ~300 ns).
"""
import numpy as np

import concourse.bacc as bacc
import concourse.mybir as mybir
from concourse.tile import TileContext
from concourse import bass_utils

M = 64        # tokens
N = 8192      # in features
K = 8192      # out features
GS = 64       # quant group size
NG = N // GS  # 128 groups
N_CORES = 8
KL = K // N_CORES   # 1024 out cols of y per core
F16 = mybir.dt.float16
F32 = mybir.dt.float32
I8 = mybir.dt.int8

NTIL = 64           # n-tiles: tile t covers n = p*64 + t, p in [0,128)
# Producer per homogeneous tile PAIR, in consumption order. The steady
# write-fabric demand must stay under the PE streaming floor (~27.5 us),
# so most tiles ride the 1 B/elem raw-int8 path ('v' = VectorE cast,
# 0.62 us/tile; 'a' = ScalarE cast, ~1.04 us/tile) and only a few use
# the 2 B/elem SWDGE cast-DMA ('c').
PAIRS = ["v", "v", "v", "a"]
for _i in range(28):
    if _i in (9, 19):
        PAIRS.append("c")
    elif _i % 3 == 1:
        PAIRS.append("a")
    else:
        PAIRS.append("v")
TILE_KIND = [k for k in PAIRS for _ in range(2)]
assert len(TILE_KIND) == NTIL
# alt (v/a) tiles per DMA chunk with issuing queue, in stream order.
# One HWDGE ring tops out at ~174 GB/s (descriptor-gen bound), so the
# alt stream is spread over all three rings: sync + scalar (HWDGE) and
# gpsimd (SWDGE raw copy, ~400 GB/s); the sync ring's head carries the
# latency-critical first chunks.
A_CHUNKS = [(4, "sync"), (4, "sync"), (6, "scalar"), (6, "gpsimd"),
            (6, "sync"), (6, "scalar"), (6, "gpsimd"), (6, "sync"),
            (6, "scalar"), (6, "gpsimd"), (4, "scalar")]
C_CHUNKS = [2, 2]
# x' pieces (tiles, queue). The stationary operand must be 128 columns
# (64-column LDWEIGHTS disables Fast Weight Load and serializes with the
# matmuls, ~300 ns/MM instead of ~215 ns), so lhsT for tile t is the
# OVERLAPPING window xT[:, t*64 : t*64+128] — the upper 64 columns are
# tile t+1's x and only produce PSUM rows 64..127, which are discarded
# at evacuation. One zero tile pads the end; pieces overlap by 64 cols.
XT_PIECES = [(2, "scalar"), (62, "gpsimd")]

_N_C = TILE_KIND.count("c")
_N_ALT = NTIL - _N_C
assert sum(n for n, _ in A_CHUNKS) == _N_ALT and sum(C_CHUNKS) == _N_C

_compiled = None


def _build():
    nc = bacc.Bacc("TRN2", target_bir_lowering=False)

    d_e = nc.declare_dram_parameter("e", [128, _N_C * KL], I8, isOutput=False)
    d_a = nc.declare_dram_parameter("a", [128, _N_ALT * KL], I8, isOutput=False)
    d_xt = nc.declare_dram_parameter("xt", [128, (NTIL + 1) * M], F16,
                                     isOutput=False)
    d_y = nc.declare_dram_parameter("y", [M, KL], F32, isOutput=True)

    with TileContext(nc) as tc:
        with (
            tc.tile_pool(name="const", bufs=1) as constp,
            tc.tile_pool(name="stagec", bufs=3) as stagec,
            tc.tile_pool(name="stagea", bufs=6) as stagea,
            tc.tile_pool(name="altf", bufs=12) as altf,
            tc.tile_pool(name="psum_y", bufs=1, space="PSUM") as psumy,
        ):
            # separate tiles per x' piece: Tile tracks deps per tile, so a
            # single split-DMA'd xT would gate the first matmul on the
            # last piece
            # piece for tiles [base, base+ntx) holds cols
            # [base*64, (base+ntx)*64 + 64) — 64-col overlap at the seam
            xt_tiles = []
            t0x = 0
            for pi, (ntx, _) in enumerate(XT_PIECES):
                xt_tiles.append(
                    (t0x, constp.tile([128, ntx * M + M], F16, name=f"xt{pi}",
                                      tag=f"xt{pi}")))
                t0x += ntx
            assert t0x == NTIL

            def xt_slice(t):
                for base, tile in reversed(xt_tiles):
                    if t >= base:
                        u = t - base
                        return tile[:, u * M:u * M + 2 * M]
                raise AssertionError

            y0 = psumy.tile([128, 512], F32, tag="y0")
            y1 = psumy.tile([128, 512], F32, tag="y1")

            # PE sits idle ~11 us while the first weight chunks stream in,
            # so the HAM clock gate re-throttles it to 1.2 GHz and the
            # first ~16 real matmuls would run at ~427 ns instead of 215.
            # Warm it with dummy matmuls on a zeroed tile during the DMA
            # head (their only dep is the memset, so they issue right
            # after the preamble).
            warm = constp.tile([128, 512], F16)
            nc.vector.memset(warm[:], 0.0)
            y_warm = psumy.tile([128, 512], F32, tag="yw")
            for wi in range(10):
                nc.tensor.matmul(
                    y_warm[:], lhsT=warm[:, 0:128], rhs=warm[:],
                    start=(wi == 0), stop=(wi == 9),
                )

            # alt / cast chunk iterators: (buffer, first-tile-covered)
            a_iter = iter(A_CHUNKS)
            c_iter = iter(C_CHUNKS)
            a_buf = a_len = a_used = a_pos = 0
            c_buf = c_len = c_used = c_pos = 0

            max_a = max(n for n, _ in A_CHUNKS)

            def next_a_chunk():
                nonlocal a_buf, a_len, a_used, a_pos
                a_len, q = next(a_iter)
                a_buf = stagea.tile([128, max_a * KL], I8, tag="a8")
                getattr(nc, q).dma_start(
                    out=a_buf[:, :a_len * KL],
                    in_=d_a[:, a_pos * KL:(a_pos + a_len) * KL],
                )
                a_pos += a_len
                a_used = 0

            def next_c_chunk():
                nonlocal c_buf, c_len, c_used, c_pos
                c_len = next(c_iter)
                c_buf = stagec.tile([128, max(C_CHUNKS) * KL], F16, tag="ec")
                nc.gpsimd.dma_start(
                    out=c_buf[:, :c_len * KL],
                    in_=d_e[:, c_pos * KL:(c_pos + c_len) * KL],
                )
                c_pos += c_len
                c_used = 0

            # head: first weight chunk leads the sync ring; the first two
            # x' pieces ride the other rings in parallel, the last piece
            # is emitted mid-loop so early alt chunks lead it on its ring
            next_a_chunk()

            def emit_xt_piece(pi):
                ntx, q = XT_PIECES[pi]
                base, tile = xt_tiles[pi]
                getattr(nc, q).dma_start(
                    out=tile[:],
                    in_=d_xt[:, base * M:(base + ntx) * M + M])

            emit_xt_piece(0)
            emit_xt_piece(1)

            for tp in range(0, NTIL, 2):        # homogeneous tile pairs
                kind = TILE_KIND[tp]
                assert TILE_KIND[tp + 1] == kind
                if tp == 4 and len(XT_PIECES) > 2:
                    emit_xt_piece(2)
                halves = None
                if kind == "c":
                    if c_used >= c_len:
                        next_c_chunk()
                    w = c_buf[:, c_used * KL:(c_used + 2) * KL]
                    c_used += 2
                elif kind == "v" and tp < 4:
                    # head: per-tile casts into per-tile buffers so the
                    # first matmuls start one cast-latency earlier (Tile
                    # deps are whole-tile; a shared buffer would gate each
                    # matmul on both casts)
                    if a_used >= a_len:
                        next_a_chunk()
                    halves = []
                    for tt in range(2):
                        src = a_buf[:, (a_used + tt) * KL:
                                    (a_used + tt + 1) * KL]
                        ht = altf.tile([128, KL], F16, tag="hf", name="hf")
                        nc.vector.tensor_copy(ht[:], src)
                        halves.append(ht)
                    a_used += 2
                    w = None
                else:
                    if a_used >= a_len:
                        next_a_chunk()
                    src = a_buf[:, a_used * KL:(a_used + 2) * KL]
                    a_used += 2
                    wt = altf.tile([128, 2 * KL], F16, tag="af")
                    if kind == "v":
                        nc.vector.tensor_copy(wt[:], src)
                    else:
                        nc.scalar.copy(wt[:], src)
                    w = wt[:]
                for tt in range(2):
                    t = tp + tt
                    if halves is not None:
                        r0 = halves[tt][:, 0:512]
                        r1 = halves[tt][:, 512:1024]
                    else:
                        r0 = w[:, tt * KL:tt * KL + 512]
                        r1 = w[:, tt * KL + 512:(tt + 1) * KL]
                    nc.tensor.matmul(
                        y0[:], lhsT=xt_slice(t), rhs=r0,
                        start=(t == 0), stop=(t == NTIL - 1),
                    )
                    nc.tensor.matmul(
                        y1[:], lhsT=xt_slice(t), rhs=r1,
                        start=(t == 0), stop=(t == NTIL - 1),
                    )

            y_sb = constp.tile([M, KL], F32)
            nc.vector.tensor_copy(y_sb[:, 0:512], y0[0:M, :])
            nc.scalar.copy(y_sb[:, 512:1024], y1[0:M, :])
            nc.sync.dma_start(out=d_y[:], in_=y_sb[:])

    nc.compile()
    return nc


def _get_compiled():
    global _compiled
    if _compiled is None:
        _compiled = _build()
    return _compiled


def _prep(x, W_q, scales, zeros, mask, mu1, mu2, bias):
    x = np.asarray(x, dtype=np.float32)
    W_q = np.asarray(W_q).astype(np.int8, copy=False)
    scales = np.asarray(scales, dtype=np.float32).reshape(K, NG)
    zeros = np.asarray(zeros, dtype=np.float32).reshape(K, NG)
    mask = np.asarray(mask, dtype=np.float32)
    mu1 = np.asarray(mu1, dtype=np.float32)
    mu2 = np.asarray(mu2, dtype=np.float32)
    bias = np.asarray(bias, dtype=np.float32)

    # v = full dequant except mu1; recode as per-row int8
    q = W_q.astype(np.float32).reshape(K, NG, GS)
    v = (q - zeros[:, :, None]) * (scales * mu2[:, None])[:, :, None]
    v = v.reshape(K, N)
    v *= mask
    d = np.abs(v).max(axis=1) / 127.0
    e8 = np.rint(v * (1.0 / d)[:, None]).astype(np.int8)

    # x' = x * mu1, f16, permuted [p, t, m] with n = p*64 + t, plus one
    # zero tile at the end (lhsT windows overlap into tile t+1)
    xp = (x * mu1[None, :]).astype(np.float16)
    xtp = np.zeros((128, NTIL + 1, M), dtype=np.float16)
    xtp[:, :NTIL, :] = xp.reshape(M, 128, NTIL).transpose(1, 2, 0)
    xtp = xtp.reshape(128, (NTIL + 1) * M)

    cast_tiles = [t for t in range(NTIL) if TILE_KIND[t] == "c"]
    alt_tiles = [t for t in range(NTIL) if TILE_KIND[t] != "c"]

    in_maps = []
    for c in range(N_CORES):
        r = slice(c * KL, (c + 1) * KL)
        # e8[r]: [KL, N] -> [p, t, k] with n = p*64 + t
        et = np.ascontiguousarray(
            e8[r].reshape(KL, 128, NTIL).transpose(1, 2, 0))  # [128, NTIL, KL]
        e_core = np.ascontiguousarray(et[:, cast_tiles, :]).reshape(128, -1)
        a_core = np.ascontiguousarray(et[:, alt_tiles, :]).reshape(128, -1)
        in_maps.append({"e": e_core, "a": a_core, "xt": xtp})
    return in_maps, d, bias


def kernel(x, W_q, scales, zeros, mask, mu1, mu2, bias, **run_kwargs):
    nc = _get_compiled()
    in_maps, d, bias_f = _prep(x, W_q, scales, zeros, mask, mu1, mu2, bias)
    res = bass_utils.run_bass_kernel_spmd(
        nc, in_maps, core_ids=list(range(N_CORES)), **run_kwargs
    )
    y = np.concatenate([res.results[c]["y"] for c in range(N_CORES)], axis=1)
    y = y * d[None, :] + bias_f[None, :]
    if run_kwargs:
        return y, res
    return y


# revision 60
# speedup vs baseline: 1.0604x; 1.0604x over previous
"""BCP quantized linear SPMD kernel for 8 Trainium2 NeuronCores.

Computes y = x @ W_deq.T + bias where
  W_deq = ((W_q - zeros) * scales) * mu2[:,None] * mu1[None,:] * mask

Sharding: tensor-parallel along the output dim K (8192 -> 1024 rows/core).
x is replicated; the [64, 1024] per-core outputs are concatenated on the
host.

The host folds the entire dequant into an int8 recode of the weight:
  v[k,n]  = (W_q - zeros) * scales * mu2 * mask          (mu1 folds into x)
  d[k]    = max_n |v[k,n]| / 127
  e8[k,n] = rint(v[k,n] / d[k])                          (int8)
so on device y_raw = x' @ e8.T is a single f16 matmul over the int8
stream, and the host applies the per-row scale d[k] and bias to the
gathered output.

The int8 -> f16 up-conversion is spread over three producers so no
single resource binds (PE streaming floor is ~27.5 us):
  'c' tiles: SWDGE cast-DMA (2 B/elem SBUF writes),
  'v' tiles: raw int8 DMA + VectorE copy-cast (~0.62 us/tile),
  'a' tiles: raw int8 DMA + ScalarE copy-cast (~1.04 us/tile).
DMA chunks are >=4 tiles so per-partition rows are >=4 KB (small rows
are descriptor-generation-bound); casts run at 2-tile granularity.
Each tile t contributes two accumulating matmuls (PSUM [128, 512] x2,
rows 64..127 discarded) with lhsT the overlapping 128-column window
x'T[:, t*64 : t*64+128] (n permuted as n = p*64 + t), which keeps Fast
Weight Load enabled (~215 ns/MM instead of ~300 ns).
"""
import numpy as np

import concourse.bacc as bacc
import concourse.mybir as mybir
from concourse.tile import TileContext
from concourse import bass_utils

M = 64        # tokens
N = 8192      # in features
K = 8192      # out features
GS = 64       # quant group size
NG = N // GS  # 128 groups
N_CORES = 8
KL = K // N_CORES   # 1024 out cols of y per core
F16 = mybir.dt.float16
F32 = mybir.dt.float32
I8 = mybir.dt.int8

NTIL = 64           # n-tiles: tile t covers n = p*64 + t, p in [0,128)
# Producer per homogeneous tile PAIR, in consumption order. The steady
# write-fabric demand must stay under the PE streaming floor (~27.5 us),
# so most tiles ride the 1 B/elem raw-int8 path ('v' = VectorE cast,
# 0.62 us/tile; 'a' = ScalarE cast, ~1.04 us/tile) and only a few use
# the 2 B/elem SWDGE cast-DMA ('c').
PAIRS = ["v", "v", "v", "a"]
for _i in range(28):
    if _i in (9, 19):
        PAIRS.append("c")
    elif _i % 3 == 1:
        PAIRS.append("a")
    else:
        PAIRS.append("v")
TILE_KIND = [k for k in PAIRS for _ in range(2)]
assert len(TILE_KIND) == NTIL
# alt (v/a) tiles per DMA chunk with issuing queue, in stream order.
# One HWDGE ring tops out at ~174 GB/s (descriptor-gen bound), so the
# alt stream is spread over all three rings: sync + scalar (HWDGE) and
# gpsimd (SWDGE raw copy, ~400 GB/s); the sync ring's head carries the
# latency-critical first chunks.
A_CHUNKS = [(4, "sync"), (4, "sync"), (6, "scalar"), (6, "gpsimd"),
            (6, "sync"), (6, "scalar"), (6, "gpsimd"), (6, "sync"),
            (6, "scalar"), (6, "gpsimd"), (4, "scalar")]
C_CHUNKS = [2, 2]
# x' pieces (tiles, queue). The stationary operand must be 128 columns
# (64-column LDWEIGHTS disables Fast Weight Load and serializes with the
# matmuls, ~300 ns/MM instead of ~215 ns), so lhsT for tile t is the
# OVERLAPPING window xT[:, t*64 : t*64+128] — the upper 64 columns are
# tile t+1's x and only produce PSUM rows 64..127, which are discarded
# at evacuation. One zero tile pads the end; pieces overlap by 64 cols.
XT_PIECES = [(2, "scalar"), (10, "scalar"), (52, "gpsimd")]

_N_C = TILE_KIND.count("c")
_N_ALT = NTIL - _N_C
assert sum(n for n, _ in A_CHUNKS) == _N_ALT and sum(C_CHUNKS) == _N_C

_compiled = None


def _build():
    nc = bacc.Bacc("TRN2", target_bir_lowering=False)

    d_e = nc.declare_dram_parameter("e", [128, _N_C * KL], I8, isOutput=False)
    d_a = nc.declare_dram_parameter("a", [128, _N_ALT * KL], I8, isOutput=False)
    d_xt = nc.declare_dram_parameter("xt", [128, (NTIL + 1) * M], F16,
                                     isOutput=False)
    d_y = nc.declare_dram_parameter("y", [M, KL], F32, isOutput=True)

    with TileContext(nc) as tc:
        with (
            tc.tile_pool(name="const", bufs=1) as constp,
            tc.tile_pool(name="stagec", bufs=3) as stagec,
            tc.tile_pool(name="stagea", bufs=6) as stagea,
            tc.tile_pool(name="altf", bufs=12) as altf,
            tc.tile_pool(name="psum_y", bufs=1, space="PSUM") as psumy,
        ):
            # separate tiles per x' piece: Tile tracks deps per tile, so a
            # single split-DMA'd xT would gate the first matmul on the
            # last piece
            # piece for tiles [base, base+ntx) holds cols
            # [base*64, (base+ntx)*64 + 64) — 64-col overlap at the seam
            xt_tiles = []
            t0x = 0
            for pi, (ntx, _) in enumerate(XT_PIECES):
                xt_tiles.append(
                    (t0x, constp.tile([128, ntx * M + M], F16, name=f"xt{pi}",
                                      tag=f"xt{pi}")))
                t0x += ntx
            assert t0x == NTIL

            def xt_slice(t):
                for base, tile in reversed(xt_tiles):
                    if t >= base:
                        u = t - base
                        return tile[:, u * M:u * M + 2 * M]
                raise AssertionError

            y0 = psumy.tile([128, 512], F32, tag="y0")
            y1 = psumy.tile([128, 512], F32, tag="y1")

            # alt / cast chunk iterators: (buffer, first-tile-covered)
            a_iter = iter(A_CHUNKS)
            c_iter = iter(C_CHUNKS)
            a_buf = a_len = a_used = a_pos = 0
            c_buf = c_len = c_used = c_pos = 0

            max_a = max(n for n, _ in A_CHUNKS)

            def next_a_chunk():
                nonlocal a_buf, a_len, a_used, a_pos
                a_len, q = next(a_iter)
                a_buf = stagea.tile([128, max_a * KL], I8, tag="a8")
                getattr(nc, q).dma_start(
                    out=a_buf[:, :a_len * KL],
                    in_=d_a[:, a_pos * KL:(a_pos + a_len) * KL],
                )
                a_pos += a_len
                a_used = 0

            def next_c_chunk():
                nonlocal c_buf, c_len, c_used, c_pos
                c_len = next(c_iter)
                c_buf = stagec.tile([128, max(C_CHUNKS) * KL], F16, tag="ec")
                nc.gpsimd.dma_start(
                    out=c_buf[:, :c_len * KL],
                    in_=d_e[:, c_pos * KL:(c_pos + c_len) * KL],
                )
                c_pos += c_len
                c_used = 0

            # head: first weight chunk leads the sync ring; the first two
            # x' pieces ride the other rings in parallel, the last piece
            # is emitted mid-loop so early alt chunks lead it on its ring
            next_a_chunk()

            def emit_xt_piece(pi):
                ntx, q = XT_PIECES[pi]
                base, tile = xt_tiles[pi]
                getattr(nc, q).dma_start(
                    out=tile[:],
                    in_=d_xt[:, base * M:(base + ntx) * M + M])

            emit_xt_piece(0)
            emit_xt_piece(1)

            for tp in range(0, NTIL, 2):        # homogeneous tile pairs
                kind = TILE_KIND[tp]
                assert TILE_KIND[tp + 1] == kind
                if tp == 4 and len(XT_PIECES) > 2:
                    emit_xt_piece(2)
                halves = None
                if kind == "c":
                    if c_used >= c_len:
                        next_c_chunk()
                    w = c_buf[:, c_used * KL:(c_used + 2) * KL]
                    c_used += 2
                elif kind == "v" and tp < 4:
                    # head: per-tile casts into per-tile buffers so the
                    # first matmuls start one cast-latency earlier (Tile
                    # deps are whole-tile; a shared buffer would gate each
                    # matmul on both casts)
                    if a_used >= a_len:
                        next_a_chunk()
                    halves = []
                    for tt in range(2):
                        src = a_buf[:, (a_used + tt) * KL:
                                    (a_used + tt + 1) * KL]
                        ht = altf.tile([128, KL], F16, tag="hf", name="hf")
                        nc.vector.tensor_copy(ht[:], src)
                        halves.append(ht)
                    a_used += 2
                    w = None
                else:
                    if a_used >= a_len:
                        next_a_chunk()
                    src = a_buf[:, a_used * KL:(a_used + 2) * KL]
                    a_used += 2
                    wt = altf.tile([128, 2 * KL], F16, tag="af")
                    if kind == "v":
                        nc.vector.tensor_copy(wt[:], src)
                    else:
                        nc.scalar.copy(wt[:], src)
                    w = wt[:]
                for tt in range(2):
                    t = tp + tt
                    if halves is not None:
                        r0 = halves[tt][:, 0:512]
                        r1 = halves[tt][:, 512:1024]
                    else:
                        r0 = w[:, tt * KL:tt * KL + 512]
                        r1 = w[:, tt * KL + 512:(tt + 1) * KL]
                    nc.tensor.matmul(
                        y0[:], lhsT=xt_slice(t), rhs=r0,
                        start=(t == 0), stop=(t == NTIL - 1),
                    )
                    nc.tensor.matmul(
                        y1[:], lhsT=xt_slice(t), rhs=r1,
                        start=(t == 0), stop=(t == NTIL - 1),
                    )

            y_sb = constp.tile([M, KL], F32)
            nc.vector.tensor_copy(y_sb[:, 0:512], y0[0:M, :])
            nc.scalar.copy(y_sb[:, 512:1024], y1[0:M, :])
            nc.sync.dma_start(out=d_y[:], in_=y_sb[:])

    nc.compile()
    return nc


def _get_compiled():
    global _compiled
    if _compiled is None:
        _compiled = _build()
    return _compiled


def _prep(x, W_q, scales, zeros, mask, mu1, mu2, bias):
    x = np.asarray(x, dtype=np.float32)
    W_q = np.asarray(W_q).astype(np.int8, copy=False)
    scales = np.asarray(scales, dtype=np.float32).reshape(K, NG)
    zeros = np.asarray(zeros, dtype=np.float32).reshape(K, NG)
    mask = np.asarray(mask, dtype=np.float32)
    mu1 = np.asarray(mu1, dtype=np.float32)
    mu2 = np.asarray(mu2, dtype=np.float32)
    bias = np.asarray(bias, dtype=np.float32)

    # v = full dequant except mu1; recode as per-row int8
    q = W_q.astype(np.float32).reshape(K, NG, GS)
    v = (q - zeros[:, :, None]) * (scales * mu2[:, None])[:, :, None]
    v = v.reshape(K, N)
    v *= mask
    d = np.abs(v).max(axis=1) / 127.0
    e8 = np.rint(v * (1.0 / d)[:, None]).astype(np.int8)

    # x' = x * mu1, f16, permuted [p, t, m] with n = p*64 + t, plus one
    # zero tile at the end (lhsT windows overlap into tile t+1)
    xp = (x * mu1[None, :]).astype(np.float16)
    xtp = np.zeros((128, NTIL + 1, M), dtype=np.float16)
    xtp[:, :NTIL, :] = xp.reshape(M, 128, NTIL).transpose(1, 2, 0)
    xtp = xtp.reshape(128, (NTIL + 1) * M)

    cast_tiles = [t for t in range(NTIL) if TILE_KIND[t] == "c"]
    alt_tiles = [t for t in range(NTIL) if TILE_KIND[t] != "c"]

    in_maps = []
    for c in range(N_CORES):
        r = slice(c * KL, (c + 1) * KL)
        # e8[r]: [KL, N] -> [p, t, k] with n = p*64 + t
        et = np.ascontiguousarray(
            e8[r].reshape(KL, 128, NTIL).transpose(1, 2, 0))  # [128, NTIL, KL]
        e_core = np.ascontiguousarray(et[:, cast_tiles, :]).reshape(128, -1)
        a_core = np.ascontiguousarray(et[:, alt_tiles, :]).reshape(128, -1)
        in_maps.append({"e": e_core, "a": a_core, "xt": xtp})
    return in_maps, d, bias


def kernel(x, W_q, scales, zeros, mask, mu1, mu2, bias, **run_kwargs):
    nc = _get_compiled()
    in_maps, d, bias_f = _prep(x, W_q, scales, zeros, mask, mu1, mu2, bias)
    res = bass_utils.run_bass_kernel_spmd(
        nc, in_maps, core_ids=list(range(N_CORES)), **run_kwargs
    )
    y = np.concatenate([res.results[c]["y"] for c in range(N_CORES)], axis=1)
    y = y * d[None, :] + bias_f[None, :]
    if run_kwargs:
        return y, res
    return y


# revision 61
# speedup vs baseline: 1.1172x; 1.0535x over previous
"""BCP quantized linear SPMD kernel for 8 Trainium2 NeuronCores.

Computes y = x @ W_deq.T + bias where
  W_deq = ((W_q - zeros) * scales) * mu2[:,None] * mu1[None,:] * mask

Sharding: tensor-parallel along the output dim K (8192 -> 1024 rows/core).
x is replicated; the [64, 1024] per-core outputs are concatenated on the
host.

The host folds the entire dequant into an int8 recode of the weight:
  v[k,n]  = (W_q - zeros) * scales * mu2 * mask          (mu1 folds into x)
  d[k]    = max_n |v[k,n]| / 127
  e8[k,n] = rint(v[k,n] / d[k])                          (int8)
so on device y_raw = x' @ e8.T is a single f16 matmul over the int8
stream, and the host applies the per-row scale d[k] and bias to the
gathered output.

The int8 -> f16 up-conversion is spread over three producers so no
single resource binds (PE streaming floor is ~27.5 us):
  'c' tiles: SWDGE cast-DMA (2 B/elem SBUF writes),
  'v' tiles: raw int8 DMA + VectorE copy-cast (~0.62 us/tile),
  'a' tiles: raw int8 DMA + ScalarE copy-cast (~1.04 us/tile).
DMA chunks are >=4 tiles so per-partition rows are >=4 KB (small rows
are descriptor-generation-bound); casts run at 2-tile granularity.
Each tile t contributes two accumulating matmuls (PSUM [128, 512] x2,
rows 64..127 discarded) with lhsT the overlapping 128-column window
x'T[:, t*64 : t*64+128] (n permuted as n = p*64 + t), which keeps Fast
Weight Load enabled (~215 ns/MM instead of ~300 ns).
"""
import numpy as np

import concourse.bacc as bacc
import concourse.mybir as mybir
from concourse.tile import TileContext
from concourse import bass_utils

M = 64        # tokens
N = 8192      # in features
K = 8192      # out features
GS = 64       # quant group size
NG = N // GS  # 128 groups
N_CORES = 8
KL = K // N_CORES   # 1024 out cols of y per core
F16 = mybir.dt.float16
F32 = mybir.dt.float32
I8 = mybir.dt.int8

NTIL = 64           # n-tiles: tile t covers n = p*64 + t, p in [0,128)
# Producer per homogeneous tile PAIR, in consumption order. The steady
# write-fabric demand must stay under the PE streaming floor (~27.5 us),
# so most tiles ride the 1 B/elem raw-int8 path ('v' = VectorE cast,
# 0.62 us/tile; 'a' = ScalarE cast, ~1.04 us/tile) and only a few use
# the 2 B/elem SWDGE cast-DMA ('c').
PAIRS = ["v", "v", "v", "a"]
for _i in range(28):
    if _i in (9, 19):
        PAIRS.append("c")
    elif _i % 3 == 1:
        PAIRS.append("a")
    else:
        PAIRS.append("v")
TILE_KIND = [k for k in PAIRS for _ in range(2)]
assert len(TILE_KIND) == NTIL
# alt (v/a) tiles per DMA chunk with issuing queue, in stream order.
# One HWDGE ring tops out at ~174 GB/s (descriptor-gen bound), so the
# alt stream is spread over all three rings: sync + scalar (HWDGE) and
# gpsimd (SWDGE raw copy, ~400 GB/s); the sync ring's head carries the
# latency-critical first chunks.
A_CHUNKS = [(4, "sync"), (4, "sync"), (6, "scalar"), (6, "gpsimd"),
            (6, "sync"), (6, "scalar"), (6, "gpsimd"), (6, "sync"),
            (6, "scalar"), (6, "gpsimd"), (4, "scalar")]
C_CHUNKS = [2, 2]
# x' pieces (tiles, queue). The stationary operand must be 128 columns
# (64-column LDWEIGHTS disables Fast Weight Load and serializes with the
# matmuls, ~300 ns/MM instead of ~215 ns), so lhsT for tile t is the
# OVERLAPPING window xT[:, t*64 : t*64+128] — the upper 64 columns are
# tile t+1's x and only produce PSUM rows 64..127, which are discarded
# at evacuation. One zero tile pads the end; pieces overlap by 64 cols.
XT_PIECES = [(2, "scalar"), (62, "gpsimd")]

_N_C = TILE_KIND.count("c")
_N_ALT = NTIL - _N_C
assert sum(n for n, _ in A_CHUNKS) == _N_ALT and sum(C_CHUNKS) == _N_C

_compiled = None


def _build():
    nc = bacc.Bacc("TRN2", target_bir_lowering=False)

    d_e = nc.declare_dram_parameter("e", [128, _N_C * KL], I8, isOutput=False)
    d_a = nc.declare_dram_parameter("a", [128, _N_ALT * KL], I8, isOutput=False)
    d_xt = nc.declare_dram_parameter("xt", [128, (NTIL + 1) * M], F16,
                                     isOutput=False)
    d_y = nc.declare_dram_parameter("y", [M, KL], F32, isOutput=True)

    with TileContext(nc) as tc:
        with (
            tc.tile_pool(name="const", bufs=1) as constp,
            tc.tile_pool(name="stagec", bufs=3) as stagec,
            tc.tile_pool(name="stagea", bufs=6) as stagea,
            tc.tile_pool(name="altf", bufs=12) as altf,
            tc.tile_pool(name="psum_y", bufs=1, space="PSUM") as psumy,
        ):
            # separate tiles per x' piece: Tile tracks deps per tile, so a
            # single split-DMA'd xT would gate the first matmul on the
            # last piece
            # piece for tiles [base, base+ntx) holds cols
            # [base*64, (base+ntx)*64 + 64) — 64-col overlap at the seam
            xt_tiles = []
            t0x = 0
            for pi, (ntx, _) in enumerate(XT_PIECES):
                xt_tiles.append(
                    (t0x, constp.tile([128, ntx * M + M], F16, name=f"xt{pi}",
                                      tag=f"xt{pi}")))
                t0x += ntx
            assert t0x == NTIL

            def xt_slice(t):
                for base, tile in reversed(xt_tiles):
                    if t >= base:
                        u = t - base
                        return tile[:, u * M:u * M + 2 * M]
                raise AssertionError

            y0 = psumy.tile([128, 512], F32, tag="y0")
            y1 = psumy.tile([128, 512], F32, tag="y1")

            # alt / cast chunk iterators: (buffer, first-tile-covered)
            a_iter = iter(A_CHUNKS)
            c_iter = iter(C_CHUNKS)
            a_buf = a_len = a_used = a_pos = 0
            c_buf = c_len = c_used = c_pos = 0

            max_a = max(n for n, _ in A_CHUNKS)

            def next_a_chunk():
                nonlocal a_buf, a_len, a_used, a_pos
                a_len, q = next(a_iter)
                a_buf = stagea.tile([128, max_a * KL], I8, tag="a8")
                getattr(nc, q).dma_start(
                    out=a_buf[:, :a_len * KL],
                    in_=d_a[:, a_pos * KL:(a_pos + a_len) * KL],
                )
                a_pos += a_len
                a_used = 0

            def next_c_chunk():
                nonlocal c_buf, c_len, c_used, c_pos
                c_len = next(c_iter)
                c_buf = stagec.tile([128, max(C_CHUNKS) * KL], F16, tag="ec")
                nc.gpsimd.dma_start(
                    out=c_buf[:, :c_len * KL],
                    in_=d_e[:, c_pos * KL:(c_pos + c_len) * KL],
                )
                c_pos += c_len
                c_used = 0

            # head: first weight chunk leads the sync ring; the first two
            # x' pieces ride the other rings in parallel, the last piece
            # is emitted mid-loop so early alt chunks lead it on its ring
            next_a_chunk()

            def emit_xt_piece(pi):
                ntx, q = XT_PIECES[pi]
                base, tile = xt_tiles[pi]
                getattr(nc, q).dma_start(
                    out=tile[:],
                    in_=d_xt[:, base * M:(base + ntx) * M + M])

            emit_xt_piece(0)
            emit_xt_piece(1)

            for tp in range(0, NTIL, 2):        # homogeneous tile pairs
                kind = TILE_KIND[tp]
                assert TILE_KIND[tp + 1] == kind
                if tp == 4 and len(XT_PIECES) > 2:
                    emit_xt_piece(2)
                halves = None
                if kind == "c":
                    if c_used >= c_len:
                        next_c_chunk()
                    w = c_buf[:, c_used * KL:(c_used + 2) * KL]
                    c_used += 2
                elif kind == "v" and tp < 4:
                    # head: per-tile casts into per-tile buffers so the
                    # first matmuls start one cast-latency earlier (Tile
                    # deps are whole-tile; a shared buffer would gate each
                    # matmul on both casts)
                    if a_used >= a_len:
                        next_a_chunk()
                    halves = []
                    for tt in range(2):
                        src = a_buf[:, (a_used + tt) * KL:
                                    (a_used + tt + 1) * KL]
                        ht = altf.tile([128, KL], F16, tag="hf", name="hf")
                        nc.vector.tensor_copy(ht[:], src)
                        halves.append(ht)
                    a_used += 2
                    w = None
                else:
                    if a_used >= a_len:
                        next_a_chunk()
                    src = a_buf[:, a_used * KL:(a_used + 2) * KL]
                    a_used += 2
                    wt = altf.tile([128, 2 * KL], F16, tag="af")
                    if kind == "v":
                        nc.vector.tensor_copy(wt[:], src)
                    else:
                        nc.scalar.copy(wt[:], src)
                    w = wt[:]
                for tt in range(2):
                    t = tp + tt
                    if halves is not None:
                        r0 = halves[tt][:, 0:512]
                        r1 = halves[tt][:, 512:1024]
                    else:
                        r0 = w[:, tt * KL:tt * KL + 512]
                        r1 = w[:, tt * KL + 512:(tt + 1) * KL]
                    nc.tensor.matmul(
                        y0[:], lhsT=xt_slice(t), rhs=r0,
                        start=(t == 0), stop=(t == NTIL - 1),
                    )
                    nc.tensor.matmul(
                        y1[:], lhsT=xt_slice(t), rhs=r1,
                        start=(t == 0), stop=(t == NTIL - 1),
                    )

            y_sb = constp.tile([M, KL], F32)
            nc.vector.tensor_copy(y_sb[:, 0:512], y0[0:M, :])
            nc.scalar.copy(y_sb[:, 512:1024], y1[0:M, :])
            nc.sync.dma_start(out=d_y[:], in_=y_sb[:])

    nc.compile()
    return nc


def _get_compiled():
    global _compiled
    if _compiled is None:
        _compiled = _build()
    return _compiled


def _prep(x, W_q, scales, zeros, mask, mu1, mu2, bias):
    x = np.asarray(x, dtype=np.float32)
    W_q = np.asarray(W_q).astype(np.int8, copy=False)
    scales = np.asarray(scales, dtype=np.float32).reshape(K, NG)
    zeros = np.asarray(zeros, dtype=np.float32).reshape(K, NG)
    mask = np.asarray(mask, dtype=np.float32)
    mu1 = np.asarray(mu1, dtype=np.float32)
    mu2 = np.asarray(mu2, dtype=np.float32)
    bias = np.asarray(bias, dtype=np.float32)

    # v = full dequant except mu1; recode as per-row int8
    q = W_q.astype(np.float32).reshape(K, NG, GS)
    v = (q - zeros[:, :, None]) * (scales * mu2[:, None])[:, :, None]
    v = v.reshape(K, N)
    v *= mask
    d = np.abs(v).max(axis=1) / 127.0
    e8 = np.rint(v * (1.0 / d)[:, None]).astype(np.int8)

    # x' = x * mu1, f16, permuted [p, t, m] with n = p*64 + t, plus one
    # zero tile at the end (lhsT windows overlap into tile t+1)
    xp = (x * mu1[None, :]).astype(np.float16)
    xtp = np.zeros((128, NTIL + 1, M), dtype=np.float16)
    xtp[:, :NTIL, :] = xp.reshape(M, 128, NTIL).transpose(1, 2, 0)
    xtp = xtp.reshape(128, (NTIL + 1) * M)

    cast_tiles = [t for t in range(NTIL) if TILE_KIND[t] == "c"]
    alt_tiles = [t for t in range(NTIL) if TILE_KIND[t] != "c"]

    in_maps = []
    for c in range(N_CORES):
        r = slice(c * KL, (c + 1) * KL)
        # e8[r]: [KL, N] -> [p, t, k] with n = p*64 + t
        et = np.ascontiguousarray(
            e8[r].reshape(KL, 128, NTIL).transpose(1, 2, 0))  # [128, NTIL, KL]
        e_core = np.ascontiguousarray(et[:, cast_tiles, :]).reshape(128, -1)
        a_core = np.ascontiguousarray(et[:, alt_tiles, :]).reshape(128, -1)
        in_maps.append({"e": e_core, "a": a_core, "xt": xtp})
    return in_maps, d, bias


def kernel(x, W_q, scales, zeros, mask, mu1, mu2, bias, **run_kwargs):
    nc = _get_compiled()
    in_maps, d, bias_f = _prep(x, W_q, scales, zeros, mask, mu1, mu2, bias)
    res = bass_utils.run_bass_kernel_spmd(
        nc, in_maps, core_ids=list(range(N_CORES)), **run_kwargs
    )
    y = np.concatenate([res.results[c]["y"] for c in range(N_CORES)], axis=1)
    y = y * d[None, :] + bias_f[None, :]
    if run_kwargs:
        return y, res
    return y
